# revision 20
# baseline (speedup 1.0000x reference)
# Trainium2 Bass kernel for nn_AttentionStream (dense transformer block with
# relative-position attention), SPMD over 8 NeuronCores.
#
# Sharding: core c -> batch b = c//2, head-group hg = c%2 (4 heads each).
# Each core computes a row-parallel partial of the output projection for its
# batch; the host sums the two partials per batch and adds the bias.
#
# v2 design ("additive pos", one exp):
#   logits^T[r, n] = dots^T + pos_skew + del(n)*right-clamp, in PSUM fp32;
#   pos[n, j] = q~ . relF[j] (pre-exp!) -> DRAM -> skew transpose-DMA gather;
#   vector adds pos/del into the dots PSUM in place; ONE scalar-engine exp
#   per tile produces P^T in SBUF.  PV: acc[65, n] += [v|1].T @ P.
#   Far-past clamp adds 0 (relF normalized), cancels in softmax.
# Engine budget: tensor ~matmul streams only, scalar ~exp only (no DMA
# triggers), vector ~adds/casts, sync ~pos stores + gathers, gpsimd ~small
# shuffles + output stores.
import os
import sys

import numpy as np
import ml_dtypes

for _p in ("/opt/trn_rl_repo", "/root/.axon_site/_ro/trn_rl_repo"):
    if _p not in sys.path and os.path.isdir(_p):
        sys.path.append(_p)

B, N, DIM = 4, 2048, 512
H, D = 8, 64          # total heads, head dim
HPC = 4               # heads per core
INNER = H * D
MAXP = 512
SCALE = D ** -0.5
NCORES = 8
W = 1280              # padded j width; j = PAD_L + 512 - d, d = n - r
PAD_L = 128
NWIN = 4              # n-windows of 512
NRC = 16              # r-chunks of 128
JW3 = [(0, 512), (512, 512), (1024, 256)]

BF = ml_dtypes.bfloat16

_CACHE = {}


def _build_bass():
    import concourse.bass as bass
    import concourse.mybir as mybir
    import concourse.tile as tile
    from concourse import bacc

    dt = mybir.dt
    fp32 = dt.float32
    bf16 = dt.bfloat16
    EXP = mybir.ActivationFunctionType.Exp

    nc = bacc.Bacc("TRN2", target_bir_lowering=False, debug=False,
                   num_devices=NCORES)

    xT = nc.dram_tensor("xT", [DIM, N], bf16, kind="ExternalInput")
    wq = nc.dram_tensor("wq", [DIM, 256], bf16, kind="ExternalInput")
    wk = nc.dram_tensor("wk", [DIM, 256], bf16, kind="ExternalInput")
    wv = nc.dram_tensor("wv", [DIM, 256], bf16, kind="ExternalInput")
    relT = nc.dram_tensor("relT", [128, W], bf16, kind="ExternalInput")
    wo = nc.dram_tensor("wo", [256, DIM], bf16, kind="ExternalInput")
    dv = nc.dram_tensor("dv", [128, 1], fp32, kind="ExternalInput")
    ident = nc.dram_tensor("ident", [128, 128], bf16, kind="ExternalInput")
    outT = nc.dram_tensor("outT", [DIM, N], fp32, kind="ExternalOutput")

    from contextlib import ExitStack
    with tile.TileContext(nc) as tc, ExitStack() as ctx:
        consts = ctx.enter_context(tc.tile_pool(name="consts", bufs=1))
        work = ctx.enter_context(tc.tile_pool(name="work", bufs=3))
        stg = ctx.enter_context(tc.tile_pool(name="stg", bufs=3))
        ppool = ctx.enter_context(tc.tile_pool(name="ppool", bufs=2))
        eppool = ctx.enter_context(tc.tile_pool(name="eppool", bufs=5))
        numep = ctx.enter_context(tc.tile_pool(name="numep", bufs=2))
        pdots = ctx.enter_context(tc.tile_pool(name="pdots", bufs=2, space="PSUM"))
        pvacc = ctx.enter_context(tc.tile_pool(name="pvacc", bufs=2, space="PSUM"))
        pacc = ctx.enter_context(tc.tile_pool(name="pacc", bufs=2, space="PSUM"))
        dramp = ctx.enter_context(tc.tile_pool(name="dramp", bufs=4, space="DRAM"))

        # ---- load constants (4 queues in parallel; startup only) ------------
        xT_sb = consts.tile([128, 4, N], bf16, tag="xT_sb")
        for dc, qeng in enumerate((nc.scalar, nc.gpsimd, nc.sync, nc.scalar)):
            qeng.dma_start(
                out=xT_sb[:, dc, :],
                in_=xT.ap()[dc * 128:(dc + 1) * 128, :])
        wq_sb = consts.tile([128, 4, 256], bf16, tag="wq_sb")
        nc.scalar.dma_start(out=wq_sb, in_=wq.ap().rearrange("(c p) i -> p c i", p=128))
        wk_sb = consts.tile([128, 4, 256], bf16, tag="wk_sb")
        nc.scalar.dma_start(out=wk_sb, in_=wk.ap().rearrange("(c p) i -> p c i", p=128))
        wv_sb = consts.tile([128, 4, 256], bf16, tag="wv_sb")
        nc.scalar.dma_start(out=wv_sb, in_=wv.ap().rearrange("(c p) i -> p c i", p=128))
        relT_sb = consts.tile([128, W], bf16, tag="relT_sb")
        nc.scalar.dma_start(out=relT_sb, in_=relT.ap())
        wo_sb = consts.tile([64, HPC, DIM], bf16, tag="wo_sb")
        nc.scalar.dma_start(out=wo_sb, in_=wo.ap().rearrange("(h p) o -> p h o", p=64))
        dv_sb = consts.tile([128, 1], fp32, tag="dv_sb")
        nc.scalar.dma_start(out=dv_sb, in_=dv.ap())
        ident_sb = consts.tile([128, 128], bf16, tag="ident_sb")
        nc.scalar.dma_start(out=ident_sb, in_=ident.ap())

        # ---- projections ----------------------------------------------------
        qT_sb = consts.tile([128, 2, N], bf16, tag="qT_sb")
        kT_sb = consts.tile([128, 2, N], bf16, tag="kT_sb")
        for dst_sb, w_sb, ceng in ((qT_sb, wq_sb, nc.vector),
                                   (kT_sb, wk_sb, nc.scalar)):
            for ic in range(2):
                for nw in range(4):   # 512-wide windows
                    ps = pacc.tile([128, 512], fp32, tag="pacc")
                    for dc in range(4):
                        nc.tensor.matmul(
                            ps,
                            lhsT=w_sb[:, dc, ic * 128:(ic + 1) * 128],
                            rhs=xT_sb[:, dc, nw * 512:(nw + 1) * 512],
                            start=(dc == 0), stop=(dc == 3))
                    if ceng is nc.scalar:
                        ceng.copy(dst_sb[:, ic, nw * 512:(nw + 1) * 512], ps)
                    else:
                        ceng.tensor_copy(dst_sb[:, ic, nw * 512:(nw + 1) * 512], ps)
        # kT_del = kT + d_vec (per-partition): lhsT for far-future-clamp cols
        kT_del = consts.tile([128, 2, N], bf16, tag="kT_del")
        nc.vector.tensor_scalar_add(kT_del, kT_sb, dv_sb)
        # v in [r, d-per-head] layout with a ones column per head: [128, rc, h, 65]
        v_sb = consts.tile([128, NRC, HPC, 65], bf16, tag="v_sb")
        nc.vector.memset(v_sb[:, :, :, 64], 1.0)
        for rc in range(NRC):
            ps = pacc.tile([128, 512], fp32, tag="pacc")
            for dc in range(4):
                nc.tensor.matmul(
                    ps[:, 0:256],
                    lhsT=xT_sb[:, dc, rc * 128:(rc + 1) * 128],
                    rhs=wv_sb[:, dc, :],
                    start=(dc == 0), stop=(dc == 3))
            nc.vector.tensor_copy(
                v_sb[:, rc, :, 0:64],
                ps[:, 0:256].rearrange("p (h d) -> p h d", h=HPC))

        # ---- per-head phases ------------------------------------------------
        pos_h = [None] * HPC      # DRAM pos tables per head
        ep_h = [None] * HPC       # 4 ep quarter tiles per head
        avn_all = consts.tile([64, HPC, NWIN, 512], bf16, tag="avn_all")

        def emit_pos_block(h, nc2):
            """pos[n, j] = q~ . relF[j] for n-rows [256*nc2, 256*(nc2+1))."""
            hc, hp = h // 2, (h % 2) * 64
            pd = pos_h[h]
            stage = stg.tile([128, 2, W], bf16, tag="stage")
            for half in range(2):
                nck = 2 * nc2 + half
                for jwi, (j0, jl) in enumerate(JW3):
                    ps = pacc.tile([128, 512], fp32, tag="pacc")
                    nc.tensor.matmul(
                        ps[:, 0:jl],
                        lhsT=qT_sb[hp:hp + 64, hc, nck * 128:(nck + 1) * 128],
                        rhs=relT_sb[hp:hp + 64, j0:j0 + jl],
                        start=True, stop=True)
                    nc.vector.tensor_copy(stage[:, half, j0:j0 + jl], ps[:, 0:jl])
            dst = pd[nc2 * 256:(nc2 + 1) * 256, :].rearrange(
                "(hf p) j -> p hf j", p=128)
            nc.sync.dma_start(out=dst, in_=stage)

        def emit_pos(h):
            pos_h[h] = dramp.tile([N, W], bf16, tag="pos", name="pos_dram")
            for nc2 in range(8):
                emit_pos_block(h, nc2)

        def emit_gather(h):
            """Skew transpose-DMA: ep[u, rc, slot, c] = pos[128*s+c,
            PAD+512+128*(rc-s)+u-c] with slot = s-rc+4, for s in rc+-4."""
            pd = pos_h[h]
            eps = [eppool.tile([128, 4, 9, 128], bf16, tag="ep", name="ep_q") for _ in range(4)]
            ep_h[h] = eps
            for rc in range(NRC):
                s_lo, s_hi = max(0, rc - 4), min(NRC - 1, rc + 4)
                k = s_hi - s_lo + 1
                off = (pd.offset + 128 * s_lo * W
                       + PAD_L + 128 * (rc - s_lo) + 512)
                src = bass.AP(tensor=pd.tensor, offset=off,
                              ap=[[W - 1, 128 * k], [1, 128]])
                slot0 = s_lo - (rc - 4)
                nc.sync.dma_start(out=eps[rc // 4][:, rc % 4, slot0:slot0 + k, :],
                                  in_=src, transpose=True)

        def emit_outproj_w(w):
            for oc in range(4):
                ps = pacc.tile([128, 512], fp32, tag="pacc")
                for h in range(HPC):
                    nc.tensor.matmul(
                        ps,
                        lhsT=wo_sb[:, h, oc * 128:(oc + 1) * 128],
                        rhs=avn_all[:, h, w, :],
                        start=(h == 0), stop=(h == HPC - 1))
                o_sb = work.tile([128, 512], fp32, tag="o_sb")
                nc.vector.tensor_copy(o_sb, ps)
                nc.gpsimd.dma_start(
                    out=outT.ap()[oc * 128:(oc + 1) * 128, w * 512:(w + 1) * 512],
                    in_=o_sb)

        def emit_att(h, interleave=None):
            """interleave: optional callable(w) emitting extra work between
            windows (pos blocks of a future head)."""
            hc, hp = h // 2, (h % 2) * 64
            eps = ep_h[h]
            num_all = numep.tile([65, NWIN, 512], fp32, tag="num_all")
            recip_dram = dramp.tile([NWIN, 512], fp32, tag="recip_dram")

            for w in range(NWIN):
                if interleave is not None:
                    interleave(w)
                n0 = w * 512
                s0 = 4 * w                     # first n-half of this window
                P_sb = ppool.tile([128, NRC, 512], bf16, tag="P_sb")
                for g in range(8):             # rc pairs
                    ps = pdots.tile([128, 2, 512], fp32, tag="dots")
                    for i in range(2):
                        rc = 2 * g + i
                        # far-future clamp (s <= rc-5): those cols use
                        # lhsT = kT + d_vec, which folds in the clamp logit.
                        nd = (min(s0 + 3, rc - 5) - s0 + 1) * 128  # del cols
                        nd = max(0, min(nd, 512))
                        qrhs = qT_sb[hp:hp + 64, hc, n0:n0 + 512]
                        if nd > 0:
                            nc.tensor.matmul(
                                ps[:, i, 0:nd],
                                lhsT=kT_del[hp:hp + 64, hc, rc * 128:(rc + 1) * 128],
                                rhs=qrhs[:, 0:nd],
                                start=True, stop=True)
                        if nd < 512:
                            nc.tensor.matmul(
                                ps[:, i, nd:512],
                                lhsT=kT_sb[hp:hp + 64, hc, rc * 128:(rc + 1) * 128],
                                rhs=qrhs[:, nd:512],
                                start=True, stop=True)
                        # band adds via identity-matmul PSUM accumulation:
                        # s in [max(s0, rc-4), min(s0+3, rc+4)]
                        sa, sb = max(s0, rc - 4), min(s0 + 3, rc + 4)
                        if sb >= sa:
                            cnt = sb - sa + 1
                            c0 = (sa - s0) * 128
                            ep = eps[rc // 4]
                            epa = ep[:, rc % 4, sa - rc + 4, 0:128]
                            ep_run = bass.AP(
                                tensor=epa.tensor, offset=epa.offset,
                                ap=[list(epa.ap[0]), [1, cnt * 128]])
                            nc.tensor.matmul(
                                ps[:, i, c0:c0 + cnt * 128],
                                lhsT=ident_sb,
                                rhs=ep_run,
                                start=False, stop=True,
                                skip_group_check=True)
                        # far-past clamp: s >= rc+5 -> add 0 (skip)
                    nc.scalar.activation(P_sb[:, 2 * g:2 * g + 2, :], ps, EXP)

                # PV accumulate: acc[0:65, n] += [v|1].T @ P
                acc = pvacc.tile([128, 512], fp32, tag="pvacc")
                for rc in range(NRC):
                    nc.tensor.matmul(
                        acc[0:65, :],
                        lhsT=v_sb[:, rc, h, :],
                        rhs=P_sb[:, rc, :],
                        start=(rc == 0), stop=(rc == NRC - 1),
                        skip_group_check=True)
                nc.vector.tensor_copy(num_all[:, w, :], acc[0:65, :])
                # per-window normalization: 1/den in place on partition 64,
                # DRAM round-trip to broadcast across 64 partitions
                nc.vector.reciprocal(num_all[64:65, w, :], num_all[64:65, w, :])
                nc.gpsimd.dma_start(out=recip_dram[w:w + 1, :],
                                    in_=num_all[64:65, w, :])
                rb_bc = work.tile([64, 512], fp32, tag="rb_bc")
                rsrc = bass.AP(tensor=recip_dram.tensor,
                               offset=recip_dram.offset + w * 512,
                               ap=[[0, 64], [1, 512]])
                nc.gpsimd.dma_start(out=rb_bc, in_=rsrc)
                nc.vector.tensor_mul(avn_all[:, h, w, :],
                                     num_all[0:64, w, :], rb_bc)
                if h == HPC - 1:
                    emit_outproj_w(w)

        # pipeline: pos(0), pos(1) upfront; pos(h+2) interleaved into att(h)
        emit_pos(0)
        emit_gather(0)
        emit_pos(1)
        emit_gather(1)

        def mk_interleave(hnext):
            # emit two pos blocks of head hnext before each window
            def f(w):
                if pos_h[hnext] is None:
                    pos_h[hnext] = dramp.tile([N, W], bf16, tag="pos", name="pos_dram")
                emit_pos_block(hnext, 2 * w)
                emit_pos_block(hnext, 2 * w + 1)
            return f

        emit_att(0, interleave=mk_interleave(2))
        emit_gather(2)
        emit_att(1, interleave=mk_interleave(3))
        emit_gather(3)
        emit_att(2)
        emit_att(3)   # emits outproj per window inline

    nc.compile()
    return nc


def host_prep(x, Wq, Wkv, Wo, bo, rel_emb):
    """Build the 8 per-core input maps (all host-side prep is O(N*D))."""
    x = np.asarray(x, np.float32)
    Wq = np.asarray(Wq, np.float32)
    Wkv = np.asarray(Wkv, np.float32)
    Wo = np.asarray(Wo, np.float32)
    rel_emb = np.asarray(rel_emb, np.float32)

    # relF[j] = rel_emb[1024-jgrid] - rel_emb[1024], edge-padded; [W, 64]
    jgrid = np.clip(np.arange(W) - PAD_L, 0, 1024)
    relF = rel_emb[1024 - jgrid] - rel_emb[1024]
    relT_one = np.ascontiguousarray(relF.T)            # [64, W]
    relT_in = np.concatenate([relT_one, relT_one], axis=0).astype(BF)  # [128, W]
    d_vec = rel_emb[0] - rel_emb[1024]                 # [64] far-future clamp
    dv_in = np.concatenate([d_vec, d_vec]).reshape(128, 1).astype(np.float32)
    ident_in = np.eye(128, dtype=np.float32).astype(BF)

    in_maps = []
    for core in range(NCORES):
        b, hg = core // 2, core % 2
        sl = slice(hg * 256, (hg + 1) * 256)
        wq_s = (Wq[:, sl] * SCALE).astype(BF)
        in_maps.append({
            "xT": np.ascontiguousarray(x[b].T).astype(BF),
            "wq": wq_s,
            "wk": Wkv[:, sl].astype(BF),
            "wv": Wkv[:, 512 + hg * 256: 512 + (hg + 1) * 256].astype(BF),
            "relT": relT_in,
            "wo": Wo[sl, :].astype(BF),
            "dv": dv_in,
            "ident": ident_in,
        })
    return in_maps


def _install_ntff_hook():
    """The agent image's antenv lacks axon_hooks; synthesize it so
    run_bass_kernel_spmd(trace=True) can capture NTFF profiles."""
    import types
    try:
        if "antenv.axon_hooks" not in sys.modules:
            import antenv
            from trn_agent_boot.trn_boot import _ntff_profile_via_ctypes
            hooks = types.ModuleType("antenv.axon_hooks")
            state = {"h": _ntff_profile_via_ctypes("/opt/axon/libaxon_pjrt.so")}
            hooks.set_axon_ntff_profile_hook = lambda h: state.__setitem__("h", h)
            hooks.get_axon_ntff_profile_hook = lambda: state["h"]
            sys.modules["antenv.axon_hooks"] = hooks
            antenv.axon_hooks = hooks
        import antenv.axon_hooks as ah
        return ah.get_axon_ntff_profile_hook() is not None
    except Exception as e:
        print(f"ntff hook install failed: {e!r}")
        return False


def kernel(x, Wq, Wkv, Wo, bo, rel_emb, _trace=False):
    import concourse.bass_utils as bu
    from concourse.bass_utils import run_bass_kernel_spmd

    if "nc" not in _CACHE:
        _CACHE["nc"] = _build_bass()
    nc = _CACHE["nc"]

    in_maps = host_prep(x, Wq, Wkv, Wo, bo, rel_emb)
    kw = {}
    if _trace and _install_ntff_hook():
        bu.upload_artifacts = lambda d: d     # zero-egress: keep artifacts local
        tmpdir = "/root/problem/traces/latest"
        import shutil
        shutil.rmtree(tmpdir, ignore_errors=True)
        os.makedirs(tmpdir, exist_ok=True)
        kw = dict(trace=True, tmpdir=tmpdir)
    res = run_bass_kernel_spmd(nc, in_maps, list(range(NCORES)), **kw)
    _CACHE["last_result"] = res

    bo = np.asarray(bo, np.float32)
    out = np.empty((B, N, DIM), np.float32)
    for b in range(B):
        pT = res.results[2 * b]["outT"] + res.results[2 * b + 1]["outT"]
        out[b] = pT.T + bo[None, :]
    return out


# revision 21
# speedup vs baseline: 1.0640x; 1.0640x over previous
# Trainium2 Bass kernel for nn_AttentionStream (dense transformer block with
# relative-position attention), SPMD over 8 NeuronCores.
#
# Sharding: core c -> batch b = c//2, head-group hg = c%2 (4 heads each).
# Each core computes a row-parallel partial of the output projection for its
# batch; the host sums the two partials per batch and adds the bias.
#
# v2 design ("additive pos", one exp):
#   logits^T[r, n] = dots^T + pos_skew + del(n)*right-clamp, in PSUM fp32;
#   pos[n, j] = q~ . relF[j] (pre-exp!) -> DRAM -> skew transpose-DMA gather;
#   vector adds pos/del into the dots PSUM in place; ONE scalar-engine exp
#   per tile produces P^T in SBUF.  PV: acc[65, n] += [v|1].T @ P.
#   Far-past clamp adds 0 (relF normalized), cancels in softmax.
# Engine budget: tensor ~matmul streams only, scalar ~exp only (no DMA
# triggers), vector ~adds/casts, sync ~pos stores + gathers, gpsimd ~small
# shuffles + output stores.
import os
import sys

import numpy as np
import ml_dtypes

for _p in ("/opt/trn_rl_repo", "/root/.axon_site/_ro/trn_rl_repo"):
    if _p not in sys.path and os.path.isdir(_p):
        sys.path.append(_p)

B, N, DIM = 4, 2048, 512
H, D = 8, 64          # total heads, head dim
HPC = 4               # heads per core
INNER = H * D
MAXP = 512
SCALE = D ** -0.5
NCORES = 8
W = 1280              # padded j width; j = PAD_L + 512 - d, d = n - r
PAD_L = 128
NWIN = 4              # n-windows of 512
NRC = 16              # r-chunks of 128
JW3 = [(0, 512), (512, 512), (1024, 256)]

BF = ml_dtypes.bfloat16

_CACHE = {}


def _build_bass():
    import concourse.bass as bass
    import concourse.mybir as mybir
    import concourse.tile as tile
    from concourse import bacc

    dt = mybir.dt
    fp32 = dt.float32
    bf16 = dt.bfloat16
    EXP = mybir.ActivationFunctionType.Exp

    nc = bacc.Bacc("TRN2", target_bir_lowering=False, debug=False,
                   num_devices=NCORES)

    xT = nc.dram_tensor("xT", [DIM, N], bf16, kind="ExternalInput")
    wq = nc.dram_tensor("wq", [DIM, 256], bf16, kind="ExternalInput")
    wk = nc.dram_tensor("wk", [DIM, 256], bf16, kind="ExternalInput")
    wv = nc.dram_tensor("wv", [DIM, 256], bf16, kind="ExternalInput")
    relT = nc.dram_tensor("relT", [128, W], bf16, kind="ExternalInput")
    wo = nc.dram_tensor("wo", [256, DIM], bf16, kind="ExternalInput")
    dv = nc.dram_tensor("dv", [128, 1], fp32, kind="ExternalInput")
    ident = nc.dram_tensor("ident", [128, 128], bf16, kind="ExternalInput")
    outT = nc.dram_tensor("outT", [DIM, N], fp32, kind="ExternalOutput")

    from contextlib import ExitStack
    with tile.TileContext(nc) as tc, ExitStack() as ctx:
        consts = ctx.enter_context(tc.tile_pool(name="consts", bufs=1))
        work = ctx.enter_context(tc.tile_pool(name="work", bufs=3))
        stg = ctx.enter_context(tc.tile_pool(name="stg", bufs=3))
        ppool = ctx.enter_context(tc.tile_pool(name="ppool", bufs=2))
        eppool = ctx.enter_context(tc.tile_pool(name="eppool", bufs=5))
        numep = ctx.enter_context(tc.tile_pool(name="numep", bufs=2))
        pdots = ctx.enter_context(tc.tile_pool(name="pdots", bufs=3, space="PSUM"))
        pacc = ctx.enter_context(tc.tile_pool(name="pacc", bufs=2, space="PSUM"))
        dramp = ctx.enter_context(tc.tile_pool(name="dramp", bufs=4, space="DRAM"))

        # ---- load constants (4 queues in parallel; startup only) ------------
        xT_sb = consts.tile([128, 4, N], bf16, tag="xT_sb")
        for dc, qeng in enumerate((nc.scalar, nc.gpsimd, nc.sync, nc.scalar)):
            qeng.dma_start(
                out=xT_sb[:, dc, :],
                in_=xT.ap()[dc * 128:(dc + 1) * 128, :])
        wq_sb = consts.tile([128, 4, 256], bf16, tag="wq_sb")
        nc.scalar.dma_start(out=wq_sb, in_=wq.ap().rearrange("(c p) i -> p c i", p=128))
        wk_sb = consts.tile([128, 4, 256], bf16, tag="wk_sb")
        nc.scalar.dma_start(out=wk_sb, in_=wk.ap().rearrange("(c p) i -> p c i", p=128))
        wv_sb = consts.tile([128, 4, 256], bf16, tag="wv_sb")
        nc.scalar.dma_start(out=wv_sb, in_=wv.ap().rearrange("(c p) i -> p c i", p=128))
        relT_sb = consts.tile([128, W], bf16, tag="relT_sb")
        nc.scalar.dma_start(out=relT_sb, in_=relT.ap())
        wo_sb = consts.tile([64, HPC, DIM], bf16, tag="wo_sb")
        nc.scalar.dma_start(out=wo_sb, in_=wo.ap().rearrange("(h p) o -> p h o", p=64))
        dv_sb = consts.tile([128, 1], fp32, tag="dv_sb")
        nc.scalar.dma_start(out=dv_sb, in_=dv.ap())
        ident_sb = consts.tile([128, 128], bf16, tag="ident_sb")
        nc.scalar.dma_start(out=ident_sb, in_=ident.ap())

        # ---- projections ----------------------------------------------------
        qT_sb = consts.tile([128, 2, N], bf16, tag="qT_sb")
        kT_sb = consts.tile([128, 2, N], bf16, tag="kT_sb")
        for dst_sb, w_sb, ceng in ((qT_sb, wq_sb, nc.vector),
                                   (kT_sb, wk_sb, nc.scalar)):
            for ic in range(2):
                for nw in range(4):   # 512-wide windows
                    ps = pacc.tile([128, 512], fp32, tag="pacc")
                    for dc in range(4):
                        nc.tensor.matmul(
                            ps,
                            lhsT=w_sb[:, dc, ic * 128:(ic + 1) * 128],
                            rhs=xT_sb[:, dc, nw * 512:(nw + 1) * 512],
                            start=(dc == 0), stop=(dc == 3))
                    if ceng is nc.scalar:
                        ceng.copy(dst_sb[:, ic, nw * 512:(nw + 1) * 512], ps)
                    else:
                        ceng.tensor_copy(dst_sb[:, ic, nw * 512:(nw + 1) * 512], ps)
        # kT_del = kT + d_vec (per-partition): lhsT for far-future-clamp cols
        kT_del = consts.tile([128, 2, N], bf16, tag="kT_del")
        nc.vector.tensor_scalar_add(kT_del, kT_sb, dv_sb)
        # v in [r, d-per-head] layout with a ones column per head: [128, rc, h, 65]
        v_sb = consts.tile([128, NRC, HPC, 65], bf16, tag="v_sb")
        nc.vector.memset(v_sb[:, :, :, 64], 1.0)
        for rc in range(NRC):
            ps = pacc.tile([128, 512], fp32, tag="pacc")
            for dc in range(4):
                nc.tensor.matmul(
                    ps[:, 0:256],
                    lhsT=xT_sb[:, dc, rc * 128:(rc + 1) * 128],
                    rhs=wv_sb[:, dc, :],
                    start=(dc == 0), stop=(dc == 3))
            nc.vector.tensor_copy(
                v_sb[:, rc, :, 0:64],
                ps[:, 0:256].rearrange("p (h d) -> p h d", h=HPC))

        # ---- per-head phases ------------------------------------------------
        pos_h = [None] * HPC      # DRAM pos tables per head
        ep_h = [None] * HPC       # 4 ep quarter tiles per head
        avn_all = consts.tile([64, HPC, NWIN, 512], bf16, tag="avn_all")

        def emit_pos_block(h, nc2):
            """pos[n, j] = q~ . relF[j] for n-rows [256*nc2, 256*(nc2+1))."""
            hc, hp = h // 2, (h % 2) * 64
            pd = pos_h[h]
            stage = stg.tile([128, 2, W], bf16, tag="stage")
            for half in range(2):
                nck = 2 * nc2 + half
                for jwi, (j0, jl) in enumerate(JW3):
                    ps = pacc.tile([128, 512], fp32, tag="pacc")
                    nc.tensor.matmul(
                        ps[:, 0:jl],
                        lhsT=qT_sb[hp:hp + 64, hc, nck * 128:(nck + 1) * 128],
                        rhs=relT_sb[hp:hp + 64, j0:j0 + jl],
                        start=True, stop=True)
                    nc.vector.tensor_copy(stage[:, half, j0:j0 + jl], ps[:, 0:jl])
            dst = pd[nc2 * 256:(nc2 + 1) * 256, :].rearrange(
                "(hf p) j -> p hf j", p=128)
            nc.sync.dma_start(out=dst, in_=stage)

        def emit_pos(h):
            pos_h[h] = dramp.tile([N, W], bf16, tag="pos", name="pos_dram")
            for nc2 in range(8):
                emit_pos_block(h, nc2)

        def emit_gather(h):
            """Skew transpose-DMA: ep[u, rc, slot, c] = pos[128*s+c,
            PAD+512+128*(rc-s)+u-c] with slot = s-rc+4, for s in rc+-4."""
            pd = pos_h[h]
            eps = [eppool.tile([128, 4, 9, 128], bf16, tag="ep", name="ep_q") for _ in range(4)]
            ep_h[h] = eps
            for rc in range(NRC):
                s_lo, s_hi = max(0, rc - 4), min(NRC - 1, rc + 4)
                k = s_hi - s_lo + 1
                off = (pd.offset + 128 * s_lo * W
                       + PAD_L + 128 * (rc - s_lo) + 512)
                src = bass.AP(tensor=pd.tensor, offset=off,
                              ap=[[W - 1, 128 * k], [1, 128]])
                slot0 = s_lo - (rc - 4)
                nc.sync.dma_start(out=eps[rc // 4][:, rc % 4, slot0:slot0 + k, :],
                                  in_=src, transpose=True)

        def emit_outproj_w(w):
            for oc in range(4):
                ps = pacc.tile([128, 512], fp32, tag="pacc")
                for h in range(HPC):
                    nc.tensor.matmul(
                        ps,
                        lhsT=wo_sb[:, h, oc * 128:(oc + 1) * 128],
                        rhs=avn_all[:, h, w, :],
                        start=(h == 0), stop=(h == HPC - 1))
                o_sb = work.tile([128, 512], fp32, tag="o_sb")
                nc.vector.tensor_copy(o_sb, ps)
                nc.gpsimd.dma_start(
                    out=outT.ap()[oc * 128:(oc + 1) * 128, w * 512:(w + 1) * 512],
                    in_=o_sb)

        def emit_att(h, interleave=None):
            """interleave: optional callable(w) emitting extra work between
            windows (pos blocks of a future head)."""
            hc, hp = h // 2, (h % 2) * 64
            eps = ep_h[h]
            num_all = numep.tile([65, NWIN, 512], fp32, tag="num_all")
            recip_dram = dramp.tile([NWIN, 512], fp32, tag="recip_dram")

            def norm_window(w):
                # 1/den in place on partition 64, DRAM round-trip to
                # broadcast across 64 partitions, then avn = num * recip
                nc.vector.reciprocal(num_all[64:65, w, :], num_all[64:65, w, :])
                nc.gpsimd.dma_start(out=recip_dram[w:w + 1, :],
                                    in_=num_all[64:65, w, :])
                rb_bc = work.tile([64, 512], fp32, tag="rb_bc")
                rsrc = bass.AP(tensor=recip_dram.tensor,
                               offset=recip_dram.offset + w * 512,
                               ap=[[0, 64], [1, 512]])
                nc.gpsimd.dma_start(out=rb_bc, in_=rsrc)
                nc.vector.tensor_mul(avn_all[:, h, w, :],
                                     num_all[0:64, w, :], rb_bc)

            for w in range(NWIN):
                if interleave is not None:
                    interleave(w)
                n0 = w * 512
                s0 = 4 * w                     # first n-half of this window
                P_sb = ppool.tile([128, NRC, 512], bf16, tag="P_sb")
                for g in range(8):             # rc pairs
                    ps = pdots.tile([128, 2, 512], fp32, tag="dots")
                    for i in range(2):
                        rc = 2 * g + i
                        # far-future clamp (s <= rc-5): those cols use
                        # lhsT = kT + d_vec, which folds in the clamp logit.
                        nd = (min(s0 + 3, rc - 5) - s0 + 1) * 128  # del cols
                        nd = max(0, min(nd, 512))
                        qrhs = qT_sb[hp:hp + 64, hc, n0:n0 + 512]
                        if nd > 0:
                            nc.tensor.matmul(
                                ps[:, i, 0:nd],
                                lhsT=kT_del[hp:hp + 64, hc, rc * 128:(rc + 1) * 128],
                                rhs=qrhs[:, 0:nd],
                                start=True, stop=True)
                        if nd < 512:
                            nc.tensor.matmul(
                                ps[:, i, nd:512],
                                lhsT=kT_sb[hp:hp + 64, hc, rc * 128:(rc + 1) * 128],
                                rhs=qrhs[:, nd:512],
                                start=True, stop=True)
                        # band adds via identity-matmul PSUM accumulation:
                        # s in [max(s0, rc-4), min(s0+3, rc+4)]
                        sa, sb = max(s0, rc - 4), min(s0 + 3, rc + 4)
                        if sb >= sa:
                            cnt = sb - sa + 1
                            c0 = (sa - s0) * 128
                            ep = eps[rc // 4]
                            epa = ep[:, rc % 4, sa - rc + 4, 0:128]
                            ep_run = bass.AP(
                                tensor=epa.tensor, offset=epa.offset,
                                ap=[list(epa.ap[0]), [1, cnt * 128]])
                            nc.tensor.matmul(
                                ps[:, i, c0:c0 + cnt * 128],
                                lhsT=ident_sb,
                                rhs=ep_run,
                                start=False, stop=True,
                                skip_group_check=True)
                        # far-past clamp: s >= rc+5 -> add 0 (skip)
                    nc.scalar.activation(P_sb[:, 2 * g:2 * g + 2, :], ps, EXP)

                # PV accumulate: acc[0:65, n] += [v|1].T @ P
                acc = pacc.tile([128, 512], fp32, tag="pacc")
                for rc in range(NRC):
                    nc.tensor.matmul(
                        acc[0:65, :],
                        lhsT=v_sb[:, rc, h, :],
                        rhs=P_sb[:, rc, :],
                        start=(rc == 0), stop=(rc == NRC - 1),
                        skip_group_check=True)
                nc.vector.tensor_copy(num_all[:, w, :], acc[0:65, :])
                if h == HPC - 1:
                    norm_window(w)
                    emit_outproj_w(w)

            if h < HPC - 1:
                nc.vector.reciprocal(num_all[64:65, :, :], num_all[64:65, :, :])
                nc.gpsimd.dma_start(out=recip_dram, in_=num_all[64:65, :, :])
                for w in range(NWIN):
                    rb_bc = work.tile([64, 512], fp32, tag="rb_bc")
                    rsrc = bass.AP(tensor=recip_dram.tensor,
                                   offset=recip_dram.offset + w * 512,
                                   ap=[[0, 64], [1, 512]])
                    nc.gpsimd.dma_start(out=rb_bc, in_=rsrc)
                    nc.vector.tensor_mul(avn_all[:, h, w, :],
                                         num_all[0:64, w, :], rb_bc)

        # pipeline: pos(0), pos(1) upfront; pos(h+2) interleaved into att(h)
        emit_pos(0)
        emit_gather(0)
        emit_pos(1)
        emit_gather(1)

        def mk_interleave(hnext):
            # emit two pos blocks of head hnext before each window
            def f(w):
                if pos_h[hnext] is None:
                    pos_h[hnext] = dramp.tile([N, W], bf16, tag="pos", name="pos_dram")
                emit_pos_block(hnext, 2 * w)
                emit_pos_block(hnext, 2 * w + 1)
            return f

        emit_att(0, interleave=mk_interleave(2))
        emit_gather(2)
        emit_att(1, interleave=mk_interleave(3))
        emit_gather(3)
        emit_att(2)
        emit_att(3)   # emits outproj per window inline

    nc.compile()
    return nc


def host_prep(x, Wq, Wkv, Wo, bo, rel_emb):
    """Build the 8 per-core input maps (all host-side prep is O(N*D))."""
    x = np.asarray(x, np.float32)
    Wq = np.asarray(Wq, np.float32)
    Wkv = np.asarray(Wkv, np.float32)
    Wo = np.asarray(Wo, np.float32)
    rel_emb = np.asarray(rel_emb, np.float32)

    # relF[j] = rel_emb[1024-jgrid] - rel_emb[1024], edge-padded; [W, 64]
    jgrid = np.clip(np.arange(W) - PAD_L, 0, 1024)
    relF = rel_emb[1024 - jgrid] - rel_emb[1024]
    relT_one = np.ascontiguousarray(relF.T)            # [64, W]
    relT_in = np.concatenate([relT_one, relT_one], axis=0).astype(BF)  # [128, W]
    d_vec = rel_emb[0] - rel_emb[1024]                 # [64] far-future clamp
    dv_in = np.concatenate([d_vec, d_vec]).reshape(128, 1).astype(np.float32)
    ident_in = np.eye(128, dtype=np.float32).astype(BF)

    in_maps = []
    for core in range(NCORES):
        b, hg = core // 2, core % 2
        sl = slice(hg * 256, (hg + 1) * 256)
        wq_s = (Wq[:, sl] * SCALE).astype(BF)
        in_maps.append({
            "xT": np.ascontiguousarray(x[b].T).astype(BF),
            "wq": wq_s,
            "wk": Wkv[:, sl].astype(BF),
            "wv": Wkv[:, 512 + hg * 256: 512 + (hg + 1) * 256].astype(BF),
            "relT": relT_in,
            "wo": Wo[sl, :].astype(BF),
            "dv": dv_in,
            "ident": ident_in,
        })
    return in_maps


def _install_ntff_hook():
    """The agent image's antenv lacks axon_hooks; synthesize it so
    run_bass_kernel_spmd(trace=True) can capture NTFF profiles."""
    import types
    try:
        if "antenv.axon_hooks" not in sys.modules:
            import antenv
            from trn_agent_boot.trn_boot import _ntff_profile_via_ctypes
            hooks = types.ModuleType("antenv.axon_hooks")
            state = {"h": _ntff_profile_via_ctypes("/opt/axon/libaxon_pjrt.so")}
            hooks.set_axon_ntff_profile_hook = lambda h: state.__setitem__("h", h)
            hooks.get_axon_ntff_profile_hook = lambda: state["h"]
            sys.modules["antenv.axon_hooks"] = hooks
            antenv.axon_hooks = hooks
        import antenv.axon_hooks as ah
        return ah.get_axon_ntff_profile_hook() is not None
    except Exception as e:
        print(f"ntff hook install failed: {e!r}")
        return False


def kernel(x, Wq, Wkv, Wo, bo, rel_emb, _trace=False):
    import concourse.bass_utils as bu
    from concourse.bass_utils import run_bass_kernel_spmd

    if "nc" not in _CACHE:
        _CACHE["nc"] = _build_bass()
    nc = _CACHE["nc"]

    in_maps = host_prep(x, Wq, Wkv, Wo, bo, rel_emb)
    kw = {}
    if _trace and _install_ntff_hook():
        bu.upload_artifacts = lambda d: d     # zero-egress: keep artifacts local
        tmpdir = "/root/problem/traces/latest"
        import shutil
        shutil.rmtree(tmpdir, ignore_errors=True)
        os.makedirs(tmpdir, exist_ok=True)
        kw = dict(trace=True, tmpdir=tmpdir)
    res = run_bass_kernel_spmd(nc, in_maps, list(range(NCORES)), **kw)
    _CACHE["last_result"] = res

    bo = np.asarray(bo, np.float32)
    out = np.empty((B, N, DIM), np.float32)
    for b in range(B):
        pT = res.results[2 * b]["outT"] + res.results[2 * b + 1]["outT"]
        out[b] = pT.T + bo[None, :]
    return out
